# revision 8
# baseline (speedup 1.0000x reference)
"""DepthCueExtractor TRN2 kernel.

out[b,u,y,x,n] = mean_v(lfi[b,u,y,x,v]) * s_mask[b,n] * h_mask[b,n,y]
  s_mask[b,n]   = sum_{h,w} f_maps[b,h,w,n]
  h_mask[b,n,y] = colsum[b,y,n] / max_w colsum[b,w,n]
  colsum[b,w,n] = sum_h f_maps[b,h,w,n]

Sharding: 8 cores = (batch b in 0..3) x (H-half in 0..1), data-parallel on the
output. Each core reads its lfi slice plus only its 128-column W-half of
f_maps[b]; the pair (2b, 2b+1) exchanges 512B of partial colsum stats
(sum/max over its half) via an in-pair AllGather, so no f_maps bytes are read
twice. colsum is computed by pre-adding the two 128-row H-halves on the
vector engine and reducing the remaining partition dim with PE ones-matmuls;
the output phase is a single broadcast tensor_tensor multiply per 2MB tile,
streamed straight to HBM. ~94.5MB of HBM traffic per core, ~270us per the
cost model (~97% of the bandwidth roofline)."""

import numpy as np

import concourse.bass as bass
import concourse.bacc as bacc
import concourse.bass_isa as bass_isa
import concourse.mybir as mybir
import concourse.tile as tile
from concourse.bass_utils import run_bass_kernel_spmd

F32 = mybir.dt.float32

B, U, H, W, V, N = 4, 9, 256, 256, 9, 64
HY = H // 2

REPLICA_GROUPS = [[0, 1], [2, 3], [4, 5], [6, 7]]


def build_kernel_body(nc, tc, lfi_s, fm, out_s, cc_in, cc_out):
    with (
        tc.tile_pool(name="const", bufs=1) as const_pool,
        tc.tile_pool(name="fmp", bufs=2) as fm_pool,
        tc.tile_pool(name="psum", bufs=1, space="PSUM") as psum_pool,
        tc.tile_pool(name="stats", bufs=1) as stats_pool,
        tc.tile_pool(name="lfip", bufs=3) as lfi_pool,
        tc.tile_pool(name="mlfp", bufs=1) as mlf_pool,
        tc.tile_pool(name="outp", bufs=2) as out_pool,
    ):
        ones = const_pool.tile([128, 1], F32)
        nc.vector.memset(ones[:], 1.0)

        # ---- Phase A: colsum[w, n] = sum_h fm[h, w, n] for my 128 w's.
        WQ = 64  # w-chunk width (PE out base partition must be 0/32/64)
        cs_psum = psum_pool.tile([128, N], F32)
        for wq in range(128 // WQ):
            f0 = fm_pool.tile([128, WQ, N], F32, name=f"f0_{wq}", tag="f0", bufs=2)
            f1 = fm_pool.tile([128, WQ, N], F32, name=f"f1_{wq}", tag="f1", bufs=2)
            # split loads + adds into halves so each add starts as soon as its
            # half of the data has landed
            for s in range(2):
                sl = slice(wq * WQ + s * (WQ // 2), wq * WQ + (s + 1) * (WQ // 2))
                tl = slice(s * (WQ // 2), (s + 1) * (WQ // 2))
                nc.sync.dma_start(out=f0[:, tl, :], in_=fm[0:128, sl, :])
                nc.sync.dma_start(out=f1[:, tl, :], in_=fm[128:256, sl, :])
                nc.vector.tensor_add(
                    out=f0[:, tl, :], in0=f0[:, tl, :], in1=f1[:, tl, :]
                )
            for n in range(N):
                nc.tensor.matmul(
                    out=cs_psum[wq * WQ : (wq + 1) * WQ, n : n + 1],
                    lhsT=f0[:, :, n],
                    rhs=ones[:, 0:1],
                    start=True,
                    stop=True,
                )

        hp = tc.high_priority
        with hp():
            cs_sb = stats_pool.tile([128, N], F32)
            nc.vector.tensor_copy(out=cs_sb[:], in_=cs_psum[:])

        # ---- Phase A2: partial stats over my half, exchange via AllGather.
        with hp():
            red_s = stats_pool.tile([128, N], F32)
            nc.gpsimd.partition_all_reduce(
                red_s[:], cs_sb[:], 128, bass_isa.ReduceOp.add
            )
            red_m = stats_pool.tile([128, N], F32)
            nc.gpsimd.partition_all_reduce(
                red_m[:], cs_sb[:], 128, bass_isa.ReduceOp.max
            )

            pack = stats_pool.tile([1, 2 * N], F32)
            nc.vector.tensor_copy(out=pack[0:1, 0:N], in_=red_s[0:1, :])
            nc.vector.tensor_copy(out=pack[0:1, N : 2 * N], in_=red_m[0:1, :])
            nc.sync.dma_start(out=cc_in[:], in_=pack[0:1, :])

            nc.gpsimd.collective_compute(
                "AllGather",
                mybir.AluOpType.bypass,
                replica_groups=REPLICA_GROUPS,
                ins=[cc_in[:]],
                outs=[cc_out[:]],
            )

            # gathered[2, 2N] -> SBUF partition-broadcast [128, 2, 2N]
            g = stats_pool.tile([128, 2, 2 * N], F32)
            cc_b = bass.AP(
                tensor=cc_out.tensor,
                offset=cc_out.offset,
                ap=[[0, 128]] + list(cc_out.ap),
            )
            nc.sync.dma_start(out=g[:], in_=cc_b)

        # ---- Phase B setup: issue all lfi loads up front (after fm loads in
        # DMA order), and the first two V-mean reduces so DVE has work while
        # the collective completes.
        lts = []
        for u in range(U):
            lt = lfi_pool.tile([128, W, V], F32, name=f"lt{u}", tag="lt", bufs=4)
            nc.sync.dma_start(out=lt[:], in_=lfi_s[u])
            lts.append(lt)

        mlf = [
            mlf_pool.tile([128, W], F32, name=f"mlf{u}", tag=f"mlf{u}")
            for u in range(U)
        ]

        def reduce_u(u):
            nc.vector.reduce_sum(
                out=mlf[u][:], in_=lts[u][:], axis=mybir.AxisListType.X
            )

        reduce_u(0)
        reduce_u(1)

        # ---- stats finalize (waits on the collective result)
        with hp():
            s_all = stats_pool.tile([128, N], F32)
            nc.vector.tensor_add(out=s_all[:], in0=g[:, 0, 0:N], in1=g[:, 1, 0:N])
            m_all = stats_pool.tile([128, N], F32)
            nc.vector.tensor_max(
                out=m_all[:], in0=g[:, 0, N : 2 * N], in1=g[:, 1, N : 2 * N]
            )

            m9 = stats_pool.tile([128, N], F32)
            nc.vector.tensor_scalar_mul(m9[:], m_all[:], float(V))
            rec = stats_pool.tile([128, N], F32)
            nc.vector.reciprocal(out=rec[:], in_=m9[:])
            sn = stats_pool.tile([128, N], F32)
            nc.vector.tensor_mul(out=sn[:], in0=s_all[:], in1=rec[:])
            wf = stats_pool.tile([128, N], F32)
            nc.vector.tensor_mul(out=wf[:], in0=cs_sb[:], in1=sn[:])

        # ---- Phase C: out[u, y, x, n] = mlf[u][y, x] * wf[y, n], with the
        # remaining V-mean reduces interleaved between output multiplies.
        xw = 64  # 2MB output tiles: fine-grained store pipelining
        wf_b = bass.AP(
            tensor=wf.tensor, offset=wf.offset, ap=[wf.ap[0], [0, xw], wf.ap[1]]
        )
        for u in range(U):
            for xh in range(W // xw):
                ot = out_pool.tile(
                    [128, xw, N], F32, name=f"ot{u}_{xh}", tag="ot", bufs=3
                )
                msl = mlf[u][:, xh * xw : (xh + 1) * xw]
                m_b = bass.AP(
                    tensor=msl.tensor, offset=msl.offset, ap=list(msl.ap) + [[0, N]]
                )
                nc.vector.tensor_mul(out=ot[:], in0=m_b, in1=wf_b)
                nc.sync.dma_start(
                    out=out_s[u, :, xh * xw : (xh + 1) * xw, :], in_=ot[:]
                )
            if u + 2 < U:
                reduce_u(u + 2)


def build_nc():
    nc = bacc.Bacc("TRN2", target_bir_lowering=False, debug=True)
    lfi_s = nc.dram_tensor("lfi_s", [U, HY, W, V], F32, kind="ExternalInput")
    fm = nc.dram_tensor("fm", [H, HY, N], F32, kind="ExternalInput")
    out_s = nc.dram_tensor("out_s", [U, HY, W, N], F32, kind="ExternalOutput")
    cc_in = nc.dram_tensor("cc_in", [1, 2 * N], F32)
    cc_out = nc.dram_tensor("cc_out", [2, 2 * N], F32)
    with tile.TileContext(nc) as tc:
        build_kernel_body(nc, tc, lfi_s, fm, out_s, cc_in[:], cc_out[:])
    nc.compile()
    return nc


_CACHE = {}


def make_in_maps(lfi, f_maps):
    in_maps = []
    for c in range(8):
        b, half = divmod(c, 2)
        lf = np.ascontiguousarray(lfi[b, :, half * HY : (half + 1) * HY])
        fmc = np.ascontiguousarray(f_maps[b][:, half * HY : (half + 1) * HY, :])
        in_maps.append({"lfi_s": lf, "fm": fmc})
    return in_maps


def kernel(lfi, f_maps):
    lfi = np.asarray(lfi, dtype=np.float32)
    f_maps = np.asarray(f_maps, dtype=np.float32)
    if "nc" not in _CACHE:
        _CACHE["nc"] = build_nc()
    nc = _CACHE["nc"]
    res = run_bass_kernel_spmd(nc, make_in_maps(lfi, f_maps), list(range(8)))
    out = np.empty((B, U, H, W, N), np.float32)
    for c in range(8):
        b, half = divmod(c, 2)
        out[b, :, half * HY : (half + 1) * HY] = res.results[c]["out_s"]
    return out
